# revision 33
# baseline (speedup 1.0000x reference)
"""Distributed AlphaFold-style triangle multiplication ("outgoing") on 8
Trainium2 NeuronCores, written in Bass/Tile.

Structure (v2):
  P1  token-sharded LN1 + gated projections; the 2*C projection channels are
      written (k-sliced) into 12 AllToAll slice buffers; each A2A#1 slice is
      issued as soon as its tokens are done, overlapping the collective with
      the rest of P1 on the TOPSP cores.
  P3  per-channel [768x768] triangle matmuls with K=128 k-tiles (full PE
      partition use); channel-group A2A#2 slices issued inside the loop.
  P4  LN2 + output projection + gating, with group-batched layernorm stats
      (stats for 8 chunks accumulated into one PSUM tile so the scalar chain
      runs on all lanes), rank-1 PSUM accumulation for the -mu*wso term, and
      bf16 output.
Key cost choices: all matmuls bf16 (1 cyc/row); LN stat rows are written to
per-group PSUM tiles at partition offsets so the rstd chain is amortized;
DMA issue load is spread across SP/ACT/Pool queues; elementwise work is
balanced across DVE/ACT/Pool.
"""
import sys
sys.path.insert(0, "/opt/trn_rl_repo")
import numpy as np
import ml_dtypes
from contextlib import ExitStack

import concourse.bass as bass
import concourse.tile as tile
from concourse import mybir
from concourse.bass_utils import run_bass_kernel_spmd

NCORES = 8
N = 768
C = 128
TB = N // NCORES            # 96 t2-rows per rank
TOK = N * TB                # 73728 tokens per rank
CH1 = 512                   # P1 chunk tokens
NCH1 = TOK // CH1           # 144
NSL = 12                    # A2A#1 slices
SLT2 = TB // NSL            # 8 t2-rows per slice
SLCH = NCH1 // NSL          # 12 P1 chunks per slice
G1 = 16                     # P1 layernorm stats group (chunks)
NKT = 6                     # P3 k-tiles of 128
CPG = 4                     # P3/P4 channels per group
NCG = 4                     # channel groups (A2A#2 slices)
CH4 = 384                   # P4 chunk tokens
NCH4 = TOK // CH4           # 192
G4 = 16                     # P4 stats group (chunks)
dt = mybir.dt
F32, BF16 = dt.float32, dt.bfloat16
AL = mybir.AluOpType
AF = mybir.ActivationFunctionType


def split_excess_waits(nc, max_waits=1):
    cnt = 0
    for fn in nc.m.functions:
        for bb in fn.blocks:
            insts = list(bb.instructions)
            out = []
            changed = False
            for inst in insts:
                si = inst.sync_info
                if si is not None and si.on_wait and len(si.on_wait) > max_waits:
                    waits = list(si.on_wait)
                    extra, keep = waits[:-max_waits], waits[-max_waits:]
                    for j in range(0, len(extra), max_waits):
                        out.append(mybir.InstNoOp(
                            name=f"{inst.name}_wsplit{j}", ins=[], outs=[],
                            sync_info=mybir.SyncInfo(on_wait=extra[j:j + max_waits], on_update=[]),
                            engine=inst.engine))
                        cnt += 1
                    si.on_wait = keep
                    changed = True
                out.append(inst)
            if changed:
                bb.instructions = out
    return cnt


def build_nc(stop_after=99):
    nc = bass.Bass("TRN2", target_bir_lowering=False, debug=False, num_devices=NCORES)

    actT = nc.declare_dram_parameter("actT", [C, TOK], BF16, isOutput=False)
    maskT = nc.declare_dram_parameter("maskT", [1, TOK], BF16, isOutput=False)
    # 5 stationary lhsT weights [c, d]: wpa, wpb, wga, wgb, wgl
    wstack = nc.declare_dram_parameter("wstack", [C, 5 * C], BF16, isOutput=False)
    woT = nc.declare_dram_parameter("woT", [C, C], BF16, isOutput=False)
    # small fp32 bias columns: [cga, cgb, cgl]
    cols = nc.declare_dram_parameter("cols", [C, 3], F32, isOutput=False)
    # one-hot column matrix, hot col 128 = 1/128: sliced per-row for stat MMs
    bigoh = nc.declare_dram_parameter("bigoh", [C, 2 * C], BF16, isOutput=False)
    # block-diagonal selectors [8, 8*128]: selc (ones) broadcasts row k of an
    # [8, T] rhs to all 128 partitions; nwsel (-wso) makes the -wso x mu rank-1
    selc = nc.declare_dram_parameter("selc", [16, 16 * C], BF16, isOutput=False)
    nwsel = nc.declare_dram_parameter("nwsel", [16, 16 * C], BF16, isOutput=False)
    outT = nc.declare_dram_parameter("outT", [C, TOK], BF16, isOutput=True)

    with tile.TileContext(nc) as tc, ExitStack() as ctx:
        dram = ctx.enter_context(tc.tile_pool(name="dram", bufs=1, space="DRAM"))
        wpool = ctx.enter_context(tc.tile_pool(name="wpool", bufs=1))

        # persistent DRAM intermediates
        p_src = dram.tile([NSL, 256, SLT2 * N], BF16, name="p_src")
        p_dst = dram.tile([NSL, 256, SLT2 * N], BF16, name="p_dst")
        # i-halved so A2A#2 (cg, ih) can fire mid-group and P4 start earlier
        tri_src = dram.tile([2, NCG, N, CPG, CH4], BF16, name="tri_src")
        tri_dst = dram.tile([2, NCG, NCORES, TB, CPG, CH4], BF16, name="tri_dst")
        gT = dram.tile([C, TOK], BF16, name="gT")

        # persistent SBUF constants
        wst = wpool.tile([C, 5 * C], BF16)
        nc.sync.dma_start(wst[:], wstack[:, :])
        wo_t = wpool.tile([C, C], BF16)
        nc.sync.dma_start(wo_t[:], woT[:, :])
        colst = wpool.tile([C, 3], F32)
        nc.sync.dma_start(colst[:], cols[:, :])
        cga, cgb, cgl = (colst[:, i:i + 1] for i in range(3))
        oh = wpool.tile([C, 2 * C], BF16)        # hot col at 128, value 1/128
        nc.sync.dma_start(oh[:], bigoh[:, :])
        sel = wpool.tile([16, 16 * C], BF16)
        nc.sync.dma_start(sel[:], selc[:, :])
        nwsl = wpool.tile([16, 16 * C], BF16)
        nc.sync.dma_start(nwsl[:], nwsel[:, :])

        def stat_mms(s12, k, G, x, sq):
            # s1 -> row k, s2 -> row 32+k of s12 (PSUM accumulate via one-hot)
            nc.tensor.matmul(s12[:], oh[:, 128 - k:256 - k], x[:],
                             start=(k == 0), stop=False)
            nc.tensor.matmul(s12[:], oh[:, 96 - k:224 - k], sq[:],
                             start=False, stop=(k == G - 1))

        def bcast_mm(out, k, row_tile):
            # out[128, T] <- row k of row_tile [8, T]
            nc.tensor.matmul(out, sel[:, C * k:C * (k + 1)], row_tile[:],
                             start=True, stop=True)

        # ---------------- Phase 1 ----------------
        NG1 = NCH1 // G1
        with tc.tile_pool(name="p1sb", bufs=2) as sb, \
             tc.tile_pool(name="p1a", bufs=G1 + 3) as sba, \
             tc.tile_pool(name="p1ch", bufs=2) as sbc, \
             tc.tile_pool(name="p1ps_s", bufs=2, space="PSUM") as ps_s, \
             tc.tile_pool(name="p1ps_b", bufs=3, space="PSUM") as ps_b, \
             tc.tile_pool(name="p1ps_p", bufs=3, space="PSUM") as ps_p:
            a16s, s12s, chains = {}, {}, {}
            for it in range(NCH1 + G1):
                # ---- stats pass for chunk ci = it ----
                if it < NCH1:
                    ci = it
                    g, k = ci // G1, ci % G1
                    t0 = ci * CH1
                    a16 = sba.tile([C, CH1], BF16, tag="a16")
                    nc.sync.dma_start(a16[:], actT[:, t0:t0 + CH1])
                    a16s[ci] = a16
                    sq16 = sb.tile([C, CH1], BF16, tag="sq16")
                    nc.gpsimd.tensor_mul(sq16[:], a16[:], a16[:])
                    if k == 0:
                        s12s[g] = ps_s.tile([C, CH1], F32, tag="stat", name="s12p1")
                    s12 = s12s[g]
                    stat_mms(s12, k, G1, a16, sq16)
                    # ---- group chain after last chunk of group ----
                    if k == G1 - 1:
                        ss = sbc.tile([G1, CH1], F32, tag="ss")
                        nc.scalar.activation(ss[:], s12[0:G1, :], AF.Square)
                        var = sbc.tile([G1, CH1], F32, tag="var")
                        nc.vector.tensor_tensor(var[:], s12[32:32 + G1, :], ss[:],
                                                op=AL.subtract)
                        vr = sbc.tile([G1, CH1], F32, tag="vr")
                        nc.vector.reciprocal(vr[:], var[:])
                        rstd = sbc.tile([G1, CH1], BF16, tag="rstd")
                        nc.scalar.sqrt(rstd[:], vr[:])
                        nrsm = sbc.tile([G1, CH1], BF16, tag="nrsm")
                        nc.vector.scalar_tensor_tensor(
                            nrsm[:], in0=s12[0:G1, :], scalar=-1.0, in1=rstd[:],
                            op0=AL.mult, op1=AL.mult)
                        chains[g] = (rstd, nrsm)
                # ---- apply pass for chunk ci = it - G1 ----
                if it >= G1:
                    ci = it - G1
                    g, k = ci // G1, ci % G1
                    t0 = ci * CH1
                    rstd, nrsm = chains[g]
                    a16 = a16s.pop(ci)
                    mask_b = sb.tile([C, CH1], BF16, tag="mask_b")
                    nc.gpsimd.dma_start(mask_b[:], maskT[:, t0:t0 + CH1].to_broadcast((C, CH1)))

                    bc_r = ps_b.tile([C, CH1], F32, tag="bc")
                    bcast_mm(bc_r[:], k, rstd)
                    bc_n = ps_b.tile([C, CH1], F32, tag="bc")
                    bcast_mm(bc_n[:], k, nrsm)
                    bcr16 = sb.tile([C, CH1], BF16, tag="bcr16")
                    nc.scalar.copy(bcr16[:], bc_r[:])
                    bcn16 = sb.tile([C, CH1], BF16, tag="bcn16")
                    nc.scalar.copy(bcn16[:], bc_n[:])
                    t16 = sb.tile([C, CH1], BF16, tag="t16")
                    nc.vector.tensor_mul(t16[:], a16[:], bcr16[:])
                    x16 = sb.tile([C, CH1], BF16, tag="x16")
                    nc.vector.tensor_tensor(x16[:], t16[:], bcn16[:], op=AL.add)
                    xm16 = sb.tile([C, CH1], BF16, tag="xm16")
                    nc.gpsimd.tensor_mul(xm16[:], x16[:], mask_b[:])

                    pp = {}
                    for nm, wi, rhs in [("ga", 2, x16), ("gb", 3, x16), ("gl", 4, x16),
                                        ("pa", 0, xm16), ("pb", 1, xm16)]:
                        ps = ps_p.tile([C, CH1], F32, tag="proj")
                        nc.tensor.matmul(ps[:], wst[:, wi * C:(wi + 1) * C], rhs[:],
                                         start=True, stop=True)
                        pp[nm] = ps
                    sa16 = sb.tile([C, CH1], BF16, tag="sa16")
                    nc.scalar.activation(sa16[:], pp["ga"][:], AF.Sigmoid, bias=cga)
                    sb16 = sb.tile([C, CH1], BF16, tag="sb16")
                    nc.scalar.activation(sb16[:], pp["gb"][:], AF.Sigmoid, bias=cgb)
                    g16 = sb.tile([C, CH1], BF16, tag="g16")
                    nc.scalar.activation(g16[:], pp["gl"][:], AF.Sigmoid, bias=cgl)
                    pa16 = sb.tile([C, CH1], BF16, tag="pa16")
                    nc.vector.tensor_mul(pa16[:], pp["pa"][:], sa16[:])
                    pb16 = sb.tile([C, CH1], BF16, tag="pb16")
                    nc.vector.tensor_mul(pb16[:], pp["pb"][:], sb16[:])

                    # scatter into slice buffer: a-chan d -> row 32*(d//16) + d%16
                    sl, cs = ci // SLCH, ci % SLCH
                    tq0 = cs * CH1
                    dsta = p_src[sl].rearrange("(s k) t -> s k t", s=NCORES)
                    nc.sync.dma_start(dsta[:, 0:16, tq0:tq0 + CH1], pa16[:])
                    nc.gpsimd.dma_start(dsta[:, 16:32, tq0:tq0 + CH1], pb16[:])
                    nc.sync.dma_start(gT[:, t0:t0 + CH1], g16[:])

                    # A2A#1 for finished slice
                    if cs == SLCH - 1 and stop_after >= 2:
                        nc.gpsimd.collective_compute(
                            "AllToAll", AL.bypass,
                            replica_groups=[list(range(NCORES))],
                            ins=[p_src[sl].opt()], outs=[p_dst[sl].opt()])

        # ---------------- Phase 3 ----------------
        # p_dst rows: (s, ch32); k_global = 96*s + 8*sl + b ; a-side ch<16, b-side 16+
        pdv = p_dst[:].rearrange("sl (s c) t -> sl s c t", s=NCORES)

        def load_ktile(dst, kt, ch, eng):
            # dst [128, N]; k in [128*kt, 128*kt+128); channel row ch of p_dst
            k0, k1 = 128 * kt, 128 * kt + 128
            s0, s1 = k0 // TB, (k1 - 1) // TB
            for s in range(s0, s1 + 1):
                klo, khi = max(k0, TB * s), min(k1, TB * (s + 1))
                jlo, jhi = (klo - TB * s) // SLT2, (khi - TB * s) // SLT2
                eng.dma_start(dst[klo - k0:khi - k0, :], pdv[jlo:jhi, s, ch, :])

        with tc.tile_pool(name="p3ab", bufs=52) as sb3, \
             tc.tile_pool(name="p3out", bufs=4) as sb3o, \
             tc.tile_pool(name="p3ps", bufs=6, space="PSUM") as ps3:
            for cg in range(NCG if stop_after >= 3 else 0):
                c0 = CPG * cg
                ats, bts = {}, {}
                for cc in range(CPG):
                    for kt in range(NKT):
                        at = sb3.tile([128, N], BF16, tag="ab", name="at")
                        load_ktile(at, kt, c0 + cc, nc.sync)
                        bt = sb3.tile([128, N], BF16, tag="ab", name="bt")
                        load_ktile(bt, kt, 16 + c0 + cc, nc.gpsimd)
                        ats[cc, kt] = at
                        bts[cc, kt] = bt
                for ih in range(2):
                    i0 = ih * CH4
                    for cc in range(CPG):
                        for jt in range(6):
                            o16 = sb3o.tile([C, CH4], BF16, tag="o16")
                            ps = ps3.tile([C, CH4], F32, tag="tri")
                            for kt in range(NKT):
                                nc.tensor.matmul(
                                    ps[:], bts[cc, kt][:, jt * C:(jt + 1) * C],
                                    ats[cc, kt][:, i0:i0 + CH4],
                                    start=(kt == 0), stop=(kt == NKT - 1))
                            nc.vector.tensor_copy(o16[:], ps[:])
                            nc.scalar.dma_start(
                                tri_src[ih, cg, jt * C:(jt + 1) * C, cc, :], o16[:])
                    if stop_after >= 4:
                        nc.gpsimd.collective_compute(
                            "AllToAll", AL.bypass,
                            replica_groups=[list(range(NCORES))],
                            ins=[tri_src[ih, cg].opt()], outs=[tri_dst[ih, cg].opt()])

        # ---------------- Phase 4 ----------------
        NG4 = NCH4 // G4
        with tc.tile_pool(name="p4sb", bufs=3) as sb4, \
             tc.tile_pool(name="p4a", bufs=G4 + 3) as sb4a, \
             tc.tile_pool(name="p4ch", bufs=2) as sb4c, \
             tc.tile_pool(name="p4ps_s", bufs=2, space="PSUM") as ps4s, \
             tc.tile_pool(name="p4ps_b", bufs=3, space="PSUM") as ps4b, \
             tc.tile_pool(name="p4ps_o", bufs=3, space="PSUM") as ps4o:
            tri16s, g16s, s12s4, chains4 = {}, {}, {}, {}
            # all ih=0 chunks first: their tri_dst halves land before ih=1's
            ci_order = list(range(0, NCH4, 2)) + list(range(1, NCH4, 2))
            for it in range(0, (NCH4 + G4) if stop_after >= 5 else 0):
                if it < NCH4:
                    ci = ci_order[it]
                    g, k = it // G4, it % G4
                    jl = ci // 2
                    t0 = ci * CH4
                    tri16 = sb4a.tile([C, CH4], BF16, tag="tri16")
                    # partitions (cg, s, cc) matching host-side perm of woT
                    for cg, eng in ((0, nc.sync), (1, nc.scalar),
                                    (2, nc.gpsimd), (3, nc.sync)):
                        eng.dma_start(tri16[32 * cg:32 * (cg + 1), :],
                                      tri_dst[ci % 2, cg, :, jl, :, :])
                    tri16s[ci] = tri16
                    g16 = sb4a.tile([C, CH4], BF16, tag="g16")
                    nc.gpsimd.dma_start(g16[:], gT[:, t0:t0 + CH4])
                    g16s[ci] = g16
                    sq16 = sb4.tile([C, CH4], BF16, tag="sq16")
                    nc.scalar.activation(sq16[:], tri16[:], AF.Square)
                    if k == 0:
                        s12s4[g] = ps4s.tile([C, CH4], F32, tag="stat", name="s12p4")
                    s12 = s12s4[g]
                    stat_mms(s12, k, G4, tri16, sq16)
                    if k == G4 - 1:
                        ss = sb4c.tile([G4, CH4], F32, tag="ss")
                        nc.scalar.activation(ss[:], s12[0:G4, :], AF.Square)
                        var = sb4c.tile([G4, CH4], F32, tag="var")
                        nc.vector.tensor_tensor(var[:], s12[32:32 + G4, :], ss[:],
                                                op=AL.subtract)
                        vr = sb4c.tile([G4, CH4], F32, tag="vr")
                        nc.vector.reciprocal(vr[:], var[:])
                        rstd = sb4c.tile([G4, CH4], BF16, tag="rstd")
                        nc.scalar.sqrt(rstd[:], vr[:])
                        mu = sb4c.tile([G4, CH4], BF16, tag="mu")
                        nc.scalar.copy(mu[:], s12[0:G4, :])
                        chains4[g] = (rstd, mu)
                if it >= G4:
                    ci = ci_order[it - G4]
                    g, k = (it - G4) // G4, (it - G4) % G4
                    t0 = ci * CH4
                    rstd, mu = chains4[g]
                    tri16 = tri16s.pop(ci)
                    g16 = g16s.pop(ci)
                    bc_r = ps4b.tile([C, CH4], F32, tag="bc")
                    bcast_mm(bc_r[:], k, rstd)
                    pso = ps4o.tile([C, CH4], F32, tag="o")
                    nc.tensor.matmul(pso[:], wo_t[:], tri16[:], start=True, stop=False)
                    nc.tensor.matmul(pso[:], nwsl[:, C * k:C * (k + 1)], mu[:],
                                     start=False, stop=True)
                    rg = sb4.tile([C, CH4], BF16, tag="rg")
                    nc.vector.tensor_mul(rg[:], bc_r[:], g16[:])
                    of16 = sb4.tile([C, CH4], BF16, tag="of16")
                    nc.vector.tensor_mul(of16[:], pso[:], rg[:])
                    nc.scalar.dma_start(outT[:, t0:t0 + CH4], of16[:])

    split_excess_waits(nc)
    return nc


def host_prep(act, mask, ln1_w, ln1_b, w_proj, w_gate, ln2_w, ln2_b, w_out, w_gl):
    bf = ml_dtypes.bfloat16
    act = np.asarray(act, np.float32)
    mask = np.asarray(mask, np.float32)
    w1 = np.asarray(ln1_w, np.float32)
    b1 = np.asarray(ln1_b, np.float32)
    w2 = np.asarray(ln2_w, np.float32)
    b2 = np.asarray(ln2_b, np.float32)
    w_proj = np.asarray(w_proj, np.float32)
    w_gate = np.asarray(w_gate, np.float32)
    w_out = np.asarray(w_out, np.float32)
    w_gl = np.asarray(w_gl, np.float32)
    assert np.all(b1 == 0.0), "nonzero ln1_b not supported in proj path"
    assert np.all(b2 == 0.0), "nonzero ln2_b not supported in output path"

    # lhsT weights [c, d] with ln1_w folded
    def lhsT(w):
        return (w.T * w1[:, None]).astype(bf)
    wstack = np.concatenate(
        [lhsT(w_proj[:C]), lhsT(w_proj[C:]), lhsT(w_gate[:C]), lhsT(w_gate[C:]), lhsT(w_gl)],
        axis=1)
    wo_p = w_out * w2[None, :]
    woT = wo_p.T.astype(bf)
    # P4 partition p = 32g + 4s + c'' holds tri channel 16s + 4g + c''
    perm = np.empty(C, np.int64)
    for g in range(4):
        for s in range(8):
            for c2 in range(4):
                perm[32 * g + 4 * s + c2] = 16 * s + 4 * g + c2
    woT = woT[perm]
    cols = np.stack([w_gate[:C] @ b1, w_gate[C:] @ b1, w_gl @ b1], axis=1).astype(np.float32)
    bigoh = np.zeros((C, 2 * C), np.float32)
    bigoh[:, 128] = 1.0 / 128.0
    bigoh = bigoh.astype(bf)
    selc = np.zeros((16, 16 * C), np.float32)
    nwsel = np.zeros((16, 16 * C), np.float32)
    nwso = -wo_p.sum(axis=1)
    for k in range(16):
        selc[k, C * k:C * (k + 1)] = 1.0
        nwsel[k, C * k:C * (k + 1)] = nwso
    selc = selc.astype(bf)
    nwsel = nwsel.astype(bf)

    in_maps = []
    for r in range(NCORES):
        blk = act[:, TB * r:TB * (r + 1), :]        # [768 t1, 96 t2, 128 c]
        actT = np.ascontiguousarray(blk.transpose(2, 1, 0).reshape(C, TOK)).astype(bf)
        mT = np.ascontiguousarray(mask[:, TB * r:TB * (r + 1)].T.reshape(1, TOK)).astype(bf)
        in_maps.append({"actT": actT, "maskT": mT, "wstack": wstack,
                        "woT": woT, "cols": cols, "bigoh": bigoh,
                        "selc": selc, "nwsel": nwsel})
    return in_maps


def assemble(results):
    out = np.empty((N, N, C), np.float32)
    for r in range(NCORES):
        o = results[r]["outT"].astype(np.float32).reshape(C, TB, N)
        out[:, TB * r:TB * (r + 1), :] = o.transpose(2, 1, 0)
    return out


_CACHE = {}

def kernel(**inputs):
    if "nc" not in _CACHE:
        _CACHE["nc"] = build_nc()
    in_maps = host_prep(**inputs)
    r = run_bass_kernel_spmd(_CACHE["nc"], in_maps, core_ids=list(range(NCORES)))
    return assemble(r.results)


# revision 35
# speedup vs baseline: 1.7133x; 1.7133x over previous
"""Distributed AlphaFold-style triangle multiplication ("outgoing") on 8
Trainium2 NeuronCores, written in Bass/Tile.

Structure (v2):
  P1  token-sharded LN1 + gated projections; the 2*C projection channels are
      written (k-sliced) into 12 AllToAll slice buffers; each A2A#1 slice is
      issued as soon as its tokens are done, overlapping the collective with
      the rest of P1 on the TOPSP cores.
  P3  per-channel [768x768] triangle matmuls with K=128 k-tiles (full PE
      partition use); channel-group A2A#2 slices issued inside the loop.
  P4  LN2 + output projection + gating, with group-batched layernorm stats
      (stats for 8 chunks accumulated into one PSUM tile so the scalar chain
      runs on all lanes), rank-1 PSUM accumulation for the -mu*wso term, and
      bf16 output.
Key cost choices: all matmuls bf16 (1 cyc/row); LN stat rows are written to
per-group PSUM tiles at partition offsets so the rstd chain is amortized;
DMA issue load is spread across SP/ACT/Pool queues; elementwise work is
balanced across DVE/ACT/Pool.
"""
import sys
sys.path.insert(0, "/opt/trn_rl_repo")
import numpy as np
import ml_dtypes
from contextlib import ExitStack

import concourse.bass as bass
import concourse.tile as tile
from concourse import mybir
from concourse.bass_utils import run_bass_kernel_spmd

NCORES = 8
N = 768
C = 128
TB = N // NCORES            # 96 t2-rows per rank
TOK = N * TB                # 73728 tokens per rank
CH1 = 512                   # P1 chunk tokens
NCH1 = TOK // CH1           # 144
NSL = 12                    # A2A#1 slices
SLT2 = TB // NSL            # 8 t2-rows per slice
SLCH = NCH1 // NSL          # 12 P1 chunks per slice
G1 = 16                     # P1 layernorm stats group (chunks)
NKT = 6                     # P3 k-tiles of 128
CPG = 4                     # P3/P4 channels per group
NCG = 4                     # channel groups (A2A#2 slices)
CH4 = 384                   # P4 chunk tokens
NCH4 = TOK // CH4           # 192
G4 = 16                     # P4 stats group (chunks)
dt = mybir.dt
F32, BF16 = dt.float32, dt.bfloat16
AL = mybir.AluOpType
AF = mybir.ActivationFunctionType


def split_excess_waits(nc, max_waits=1):
    cnt = 0
    for fn in nc.m.functions:
        for bb in fn.blocks:
            insts = list(bb.instructions)
            out = []
            changed = False
            for inst in insts:
                si = inst.sync_info
                if si is not None and si.on_wait and len(si.on_wait) > max_waits:
                    waits = list(si.on_wait)
                    extra, keep = waits[:-max_waits], waits[-max_waits:]
                    for j in range(0, len(extra), max_waits):
                        out.append(mybir.InstNoOp(
                            name=f"{inst.name}_wsplit{j}", ins=[], outs=[],
                            sync_info=mybir.SyncInfo(on_wait=extra[j:j + max_waits], on_update=[]),
                            engine=inst.engine))
                        cnt += 1
                    si.on_wait = keep
                    changed = True
                out.append(inst)
            if changed:
                bb.instructions = out
    return cnt


def build_nc(stop_after=99):
    nc = bass.Bass("TRN2", target_bir_lowering=False, debug=False, num_devices=NCORES)

    actT = nc.declare_dram_parameter("actT", [C, TOK], BF16, isOutput=False)
    maskT = nc.declare_dram_parameter("maskT", [1, TOK], BF16, isOutput=False)
    # 5 stationary lhsT weights [c, d]: wpa, wpb, wga, wgb, wgl
    wstack = nc.declare_dram_parameter("wstack", [C, 5 * C], BF16, isOutput=False)
    woT = nc.declare_dram_parameter("woT", [C, C], BF16, isOutput=False)
    # small fp32 bias columns: [cga, cgb, cgl]
    cols = nc.declare_dram_parameter("cols", [C, 3], F32, isOutput=False)
    # one-hot column matrix, hot col 128 = 1/128: sliced per-row for stat MMs
    bigoh = nc.declare_dram_parameter("bigoh", [C, 2 * C], BF16, isOutput=False)
    # block-diagonal selectors [8, 8*128]: selc (ones) broadcasts row k of an
    # [8, T] rhs to all 128 partitions; nwsel (-wso) makes the -wso x mu rank-1
    selc = nc.declare_dram_parameter("selc", [16, 16 * C], BF16, isOutput=False)
    nwsel = nc.declare_dram_parameter("nwsel", [16, 16 * C], BF16, isOutput=False)
    outT = nc.declare_dram_parameter("outT", [C, TOK], BF16, isOutput=True)

    with tile.TileContext(nc) as tc, ExitStack() as ctx:
        dram = ctx.enter_context(tc.tile_pool(name="dram", bufs=1, space="DRAM"))
        wpool = ctx.enter_context(tc.tile_pool(name="wpool", bufs=1))

        # persistent DRAM intermediates
        p_src = dram.tile([NSL, 256, SLT2 * N], BF16, name="p_src")
        p_dst = dram.tile([NSL, 256, SLT2 * N], BF16, name="p_dst")
        # i-halved so A2A#2 (cg, ih) can fire mid-group and P4 start earlier
        tri_src = dram.tile([2, NCG, N, CPG, CH4], BF16, name="tri_src")
        tri_dst = dram.tile([2, NCG, NCORES, TB, CPG, CH4], BF16, name="tri_dst")
        gT = dram.tile([C, TOK], BF16, name="gT")
        # DRAM staging of LN1 scale rows: broadcast back per chunk via DMA
        rn_dram = dram.tile([NCH1 // G1, 2, G1, CH1], BF16, name="rn_dram")

        # persistent SBUF constants
        wst = wpool.tile([C, 5 * C], BF16)
        nc.sync.dma_start(wst[:], wstack[:, :])
        wo_t = wpool.tile([C, C], BF16)
        nc.sync.dma_start(wo_t[:], woT[:, :])
        colst = wpool.tile([C, 3], F32)
        nc.sync.dma_start(colst[:], cols[:, :])
        cga, cgb, cgl = (colst[:, i:i + 1] for i in range(3))
        oh = wpool.tile([C, 2 * C], BF16)        # hot col at 128, value 1/128
        nc.sync.dma_start(oh[:], bigoh[:, :])
        sel = wpool.tile([16, 16 * C], BF16)
        nc.sync.dma_start(sel[:], selc[:, :])
        nwsl = wpool.tile([16, 16 * C], BF16)
        nc.sync.dma_start(nwsl[:], nwsel[:, :])

        def stat_mms(s12, k, G, x, sq):
            # s1 -> row k, s2 -> row 32+k of s12 (PSUM accumulate via one-hot)
            nc.tensor.matmul(s12[:], oh[:, 128 - k:256 - k], x[:],
                             start=(k == 0), stop=False)
            nc.tensor.matmul(s12[:], oh[:, 96 - k:224 - k], sq[:],
                             start=False, stop=(k == G - 1))

        def bcast_mm(out, k, row_tile):
            # out[128, T] <- row k of row_tile [8, T]
            nc.tensor.matmul(out, sel[:, C * k:C * (k + 1)], row_tile[:],
                             start=True, stop=True)

        # ---------------- Phase 1 ----------------
        NG1 = NCH1 // G1
        LEAD = G1 + 4
        with tc.tile_pool(name="p1sb", bufs=2) as sb, \
             tc.tile_pool(name="p1a", bufs=LEAD + 3) as sba, \
             tc.tile_pool(name="p1ch", bufs=2) as sbc, \
             tc.tile_pool(name="p1ps_s", bufs=2, space="PSUM") as ps_s, \
             tc.tile_pool(name="p1ps_p", bufs=5, space="PSUM") as ps_p:
            a16s, s12s = {}, {}
            for it in range(NCH1 + LEAD):
                # ---- stats pass for chunk ci = it ----
                if it < NCH1:
                    ci = it
                    g, k = ci // G1, ci % G1
                    t0 = ci * CH1
                    a16 = sba.tile([C, CH1], BF16, tag="a16")
                    nc.sync.dma_start(a16[:], actT[:, t0:t0 + CH1])
                    a16s[ci] = a16
                    sq16 = sb.tile([C, CH1], BF16, tag="sq16")
                    nc.gpsimd.tensor_mul(sq16[:], a16[:], a16[:])
                    if k == 0:
                        s12s[g] = ps_s.tile([C, CH1], F32, tag="stat", name="s12p1")
                    s12 = s12s[g]
                    stat_mms(s12, k, G1, a16, sq16)
                    # ---- group chain after last chunk of group ----
                    if k == G1 - 1:
                        ss = sbc.tile([G1, CH1], F32, tag="ss")
                        nc.scalar.activation(ss[:], s12[0:G1, :], AF.Square)
                        var = sbc.tile([G1, CH1], F32, tag="var")
                        nc.vector.tensor_tensor(var[:], s12[32:32 + G1, :], ss[:],
                                                op=AL.subtract)
                        vr = sbc.tile([G1, CH1], F32, tag="vr")
                        nc.vector.reciprocal(vr[:], var[:])
                        rstd = sbc.tile([G1, CH1], BF16, tag="rstd")
                        nc.scalar.sqrt(rstd[:], vr[:])
                        nrsm = sbc.tile([G1, CH1], BF16, tag="nrsm")
                        nc.vector.scalar_tensor_tensor(
                            nrsm[:], in0=s12[0:G1, :], scalar=-1.0, in1=rstd[:],
                            op0=AL.mult, op1=AL.mult)
                        nc.sync.dma_start(rn_dram[g, 0], rstd[:])
                        nc.scalar.dma_start(rn_dram[g, 1], nrsm[:])
                # ---- apply pass for chunk ci = it - LEAD ----
                if it >= LEAD:
                    ci = it - LEAD
                    g, k = ci // G1, ci % G1
                    t0 = ci * CH1
                    a16 = a16s.pop(ci)
                    mask_b = sb.tile([C, CH1], BF16, tag="mask_b")
                    nc.gpsimd.dma_start(mask_b[:], maskT[:, t0:t0 + CH1].to_broadcast((C, CH1)))

                    bcr16 = sb.tile([C, CH1], BF16, tag="bcr16")
                    nc.sync.dma_start(
                        bcr16[:], rn_dram[g, 0, k:k + 1, :].to_broadcast((C, CH1)))
                    bcn16 = sb.tile([C, CH1], BF16, tag="bcn16")
                    nc.scalar.dma_start(
                        bcn16[:], rn_dram[g, 1, k:k + 1, :].to_broadcast((C, CH1)))
                    t16 = sb.tile([C, CH1], BF16, tag="t16")
                    nc.vector.tensor_mul(t16[:], a16[:], bcr16[:])
                    x16 = sb.tile([C, CH1], BF16, tag="x16")
                    nc.vector.tensor_tensor(x16[:], t16[:], bcn16[:], op=AL.add)
                    xm16 = sb.tile([C, CH1], BF16, tag="xm16")
                    nc.gpsimd.tensor_mul(xm16[:], x16[:], mask_b[:])

                    pp = {}
                    for nm, wi, rhs in [("ga", 2, x16), ("gb", 3, x16), ("gl", 4, x16),
                                        ("pa", 0, xm16), ("pb", 1, xm16)]:
                        ps = ps_p.tile([C, CH1], F32, tag="proj")
                        nc.tensor.matmul(ps[:], wst[:, wi * C:(wi + 1) * C], rhs[:],
                                         start=True, stop=True)
                        pp[nm] = ps
                    sa16 = sb.tile([C, CH1], BF16, tag="sa16")
                    nc.scalar.activation(sa16[:], pp["ga"][:], AF.Sigmoid, bias=cga)
                    sb16 = sb.tile([C, CH1], BF16, tag="sb16")
                    nc.scalar.activation(sb16[:], pp["gb"][:], AF.Sigmoid, bias=cgb)
                    g16 = sb.tile([C, CH1], BF16, tag="g16")
                    nc.scalar.activation(g16[:], pp["gl"][:], AF.Sigmoid, bias=cgl)
                    pa16 = sb.tile([C, CH1], BF16, tag="pa16")
                    nc.vector.tensor_mul(pa16[:], pp["pa"][:], sa16[:])
                    pb16 = sb.tile([C, CH1], BF16, tag="pb16")
                    nc.vector.tensor_mul(pb16[:], pp["pb"][:], sb16[:])

                    # scatter into slice buffer: a-chan d -> row 32*(d//16) + d%16
                    sl, cs = ci // SLCH, ci % SLCH
                    tq0 = cs * CH1
                    dsta = p_src[sl].rearrange("(s k) t -> s k t", s=NCORES)
                    nc.sync.dma_start(dsta[:, 0:16, tq0:tq0 + CH1], pa16[:])
                    nc.gpsimd.dma_start(dsta[:, 16:32, tq0:tq0 + CH1], pb16[:])
                    nc.sync.dma_start(gT[:, t0:t0 + CH1], g16[:])

                    # A2A#1 for finished slice
                    if cs == SLCH - 1 and stop_after >= 2:
                        nc.gpsimd.collective_compute(
                            "AllToAll", AL.bypass,
                            replica_groups=[list(range(NCORES))],
                            ins=[p_src[sl].opt()], outs=[p_dst[sl].opt()])

        # ---------------- Phase 3 ----------------
        # p_dst rows: (s, ch32); k_global = 96*s + 8*sl + b ; a-side ch<16, b-side 16+
        pdv = p_dst[:].rearrange("sl (s c) t -> sl s c t", s=NCORES)

        def load_ktile(dst, kt, ch, eng):
            # dst [128, N]; k in [128*kt, 128*kt+128); channel row ch of p_dst
            k0, k1 = 128 * kt, 128 * kt + 128
            s0, s1 = k0 // TB, (k1 - 1) // TB
            for s in range(s0, s1 + 1):
                klo, khi = max(k0, TB * s), min(k1, TB * (s + 1))
                jlo, jhi = (klo - TB * s) // SLT2, (khi - TB * s) // SLT2
                eng.dma_start(dst[klo - k0:khi - k0, :], pdv[jlo:jhi, s, ch, :])

        with tc.tile_pool(name="p3ab", bufs=52) as sb3, \
             tc.tile_pool(name="p3out", bufs=4) as sb3o, \
             tc.tile_pool(name="p3ps", bufs=6, space="PSUM") as ps3:
            for cg in range(NCG if stop_after >= 3 else 0):
                c0 = CPG * cg
                ats, bts = {}, {}
                for cc in range(CPG):
                    for kt in range(NKT):
                        at = sb3.tile([128, N], BF16, tag="ab", name="at")
                        load_ktile(at, kt, c0 + cc, nc.sync)
                        bt = sb3.tile([128, N], BF16, tag="ab", name="bt")
                        load_ktile(bt, kt, 16 + c0 + cc, nc.gpsimd)
                        ats[cc, kt] = at
                        bts[cc, kt] = bt
                for ih in range(2):
                    i0 = ih * CH4
                    for cc in range(CPG):
                        for jt in range(6):
                            o16 = sb3o.tile([C, CH4], BF16, tag="o16")
                            ps = ps3.tile([C, CH4], F32, tag="tri")
                            for kt in range(NKT):
                                nc.tensor.matmul(
                                    ps[:], bts[cc, kt][:, jt * C:(jt + 1) * C],
                                    ats[cc, kt][:, i0:i0 + CH4],
                                    start=(kt == 0), stop=(kt == NKT - 1))
                            nc.vector.tensor_copy(o16[:], ps[:])
                            nc.scalar.dma_start(
                                tri_src[ih, cg, jt * C:(jt + 1) * C, cc, :], o16[:])
                    if stop_after >= 4:
                        nc.gpsimd.collective_compute(
                            "AllToAll", AL.bypass,
                            replica_groups=[list(range(NCORES))],
                            ins=[tri_src[ih, cg].opt()], outs=[tri_dst[ih, cg].opt()])

        # ---------------- Phase 4 ----------------
        NG4 = NCH4 // G4
        with tc.tile_pool(name="p4sb", bufs=3) as sb4, \
             tc.tile_pool(name="p4a", bufs=G4 + 3) as sb4a, \
             tc.tile_pool(name="p4ch", bufs=2) as sb4c, \
             tc.tile_pool(name="p4ps_s", bufs=2, space="PSUM") as ps4s, \
             tc.tile_pool(name="p4ps_b", bufs=3, space="PSUM") as ps4b, \
             tc.tile_pool(name="p4ps_o", bufs=3, space="PSUM") as ps4o:
            tri16s, g16s, s12s4, chains4 = {}, {}, {}, {}
            # all ih=0 chunks first: their tri_dst halves land before ih=1's
            ci_order = list(range(0, NCH4, 2)) + list(range(1, NCH4, 2))
            for it in range(0, (NCH4 + G4) if stop_after >= 5 else 0):
                if it < NCH4:
                    ci = ci_order[it]
                    g, k = it // G4, it % G4
                    jl = ci // 2
                    t0 = ci * CH4
                    tri16 = sb4a.tile([C, CH4], BF16, tag="tri16")
                    # partitions (cg, s, cc) matching host-side perm of woT
                    for cg, eng in ((0, nc.sync), (1, nc.scalar),
                                    (2, nc.gpsimd), (3, nc.sync)):
                        eng.dma_start(tri16[32 * cg:32 * (cg + 1), :],
                                      tri_dst[ci % 2, cg, :, jl, :, :])
                    tri16s[ci] = tri16
                    g16 = sb4a.tile([C, CH4], BF16, tag="g16")
                    nc.gpsimd.dma_start(g16[:], gT[:, t0:t0 + CH4])
                    g16s[ci] = g16
                    sq16 = sb4.tile([C, CH4], BF16, tag="sq16")
                    nc.scalar.activation(sq16[:], tri16[:], AF.Square)
                    if k == 0:
                        s12s4[g] = ps4s.tile([C, CH4], F32, tag="stat", name="s12p4")
                    s12 = s12s4[g]
                    stat_mms(s12, k, G4, tri16, sq16)
                    if k == G4 - 1:
                        ss = sb4c.tile([G4, CH4], F32, tag="ss")
                        nc.scalar.activation(ss[:], s12[0:G4, :], AF.Square)
                        var = sb4c.tile([G4, CH4], F32, tag="var")
                        nc.vector.tensor_tensor(var[:], s12[32:32 + G4, :], ss[:],
                                                op=AL.subtract)
                        vr = sb4c.tile([G4, CH4], F32, tag="vr")
                        nc.vector.reciprocal(vr[:], var[:])
                        rstd = sb4c.tile([G4, CH4], BF16, tag="rstd")
                        nc.scalar.sqrt(rstd[:], vr[:])
                        mu = sb4c.tile([G4, CH4], BF16, tag="mu")
                        nc.scalar.copy(mu[:], s12[0:G4, :])
                        chains4[g] = (rstd, mu)
                if it >= G4:
                    ci = ci_order[it - G4]
                    g, k = (it - G4) // G4, (it - G4) % G4
                    t0 = ci * CH4
                    rstd, mu = chains4[g]
                    tri16 = tri16s.pop(ci)
                    g16 = g16s.pop(ci)
                    bc_r = ps4b.tile([C, CH4], F32, tag="bc")
                    bcast_mm(bc_r[:], k, rstd)
                    pso = ps4o.tile([C, CH4], F32, tag="o")
                    nc.tensor.matmul(pso[:], wo_t[:], tri16[:], start=True, stop=False)
                    nc.tensor.matmul(pso[:], nwsl[:, C * k:C * (k + 1)], mu[:],
                                     start=False, stop=True)
                    rg = sb4.tile([C, CH4], BF16, tag="rg")
                    nc.vector.tensor_mul(rg[:], bc_r[:], g16[:])
                    of16 = sb4.tile([C, CH4], BF16, tag="of16")
                    nc.vector.tensor_mul(of16[:], pso[:], rg[:])
                    nc.scalar.dma_start(outT[:, t0:t0 + CH4], of16[:])

    split_excess_waits(nc)
    return nc


def host_prep(act, mask, ln1_w, ln1_b, w_proj, w_gate, ln2_w, ln2_b, w_out, w_gl):
    bf = ml_dtypes.bfloat16
    act = np.asarray(act, np.float32)
    mask = np.asarray(mask, np.float32)
    w1 = np.asarray(ln1_w, np.float32)
    b1 = np.asarray(ln1_b, np.float32)
    w2 = np.asarray(ln2_w, np.float32)
    b2 = np.asarray(ln2_b, np.float32)
    w_proj = np.asarray(w_proj, np.float32)
    w_gate = np.asarray(w_gate, np.float32)
    w_out = np.asarray(w_out, np.float32)
    w_gl = np.asarray(w_gl, np.float32)
    assert np.all(b1 == 0.0), "nonzero ln1_b not supported in proj path"
    assert np.all(b2 == 0.0), "nonzero ln2_b not supported in output path"

    # lhsT weights [c, d] with ln1_w folded
    def lhsT(w):
        return (w.T * w1[:, None]).astype(bf)
    wstack = np.concatenate(
        [lhsT(w_proj[:C]), lhsT(w_proj[C:]), lhsT(w_gate[:C]), lhsT(w_gate[C:]), lhsT(w_gl)],
        axis=1)
    wo_p = w_out * w2[None, :]
    woT = wo_p.T.astype(bf)
    # P4 partition p = 32g + 4s + c'' holds tri channel 16s + 4g + c''
    perm = np.empty(C, np.int64)
    for g in range(4):
        for s in range(8):
            for c2 in range(4):
                perm[32 * g + 4 * s + c2] = 16 * s + 4 * g + c2
    woT = woT[perm]
    cols = np.stack([w_gate[:C] @ b1, w_gate[C:] @ b1, w_gl @ b1], axis=1).astype(np.float32)
    bigoh = np.zeros((C, 2 * C), np.float32)
    bigoh[:, 128] = 1.0 / 128.0
    bigoh = bigoh.astype(bf)
    selc = np.zeros((16, 16 * C), np.float32)
    nwsel = np.zeros((16, 16 * C), np.float32)
    nwso = -wo_p.sum(axis=1)
    for k in range(16):
        selc[k, C * k:C * (k + 1)] = 1.0
        nwsel[k, C * k:C * (k + 1)] = nwso
    selc = selc.astype(bf)
    nwsel = nwsel.astype(bf)

    in_maps = []
    for r in range(NCORES):
        blk = act[:, TB * r:TB * (r + 1), :]        # [768 t1, 96 t2, 128 c]
        actT = np.ascontiguousarray(blk.transpose(2, 1, 0).reshape(C, TOK)).astype(bf)
        mT = np.ascontiguousarray(mask[:, TB * r:TB * (r + 1)].T.reshape(1, TOK)).astype(bf)
        in_maps.append({"actT": actT, "maskT": mT, "wstack": wstack,
                        "woT": woT, "cols": cols, "bigoh": bigoh,
                        "selc": selc, "nwsel": nwsel})
    return in_maps


def assemble(results):
    out = np.empty((N, N, C), np.float32)
    for r in range(NCORES):
        o = results[r]["outT"].astype(np.float32).reshape(C, TB, N)
        out[:, TB * r:TB * (r + 1), :] = o.transpose(2, 1, 0)
    return out


_CACHE = {}

def kernel(**inputs):
    if "nc" not in _CACHE:
        _CACHE["nc"] = build_nc()
    in_maps = host_prep(**inputs)
    r = run_bass_kernel_spmd(_CACHE["nc"], in_maps, core_ids=list(range(NCORES)))
    return assemble(r.results)
